# revision 11
# baseline (speedup 1.0000x reference)
"""Trainium2 Bass kernel for nn_ClaimEncoder (dense_mlp).

Math (per row):
  feats = [sin/cos point-encoders (2x256), leaky number-encoders (3x128)]  -> [896]
  h   = leaky_relu(feats @ W1 + b1)   -> [512]
  out = leaky_relu(h @ W2 + b2)       -> [512]

Strategy: pure data parallel over 8 NeuronCores (16384 rows each).

Device-side design (per core, batch tiles of NB=512 columns):
  * Encoder inputs arrive via ONE fp16 partition-broadcast DMA per tile
    (all 7 value rows at once).  Sin chunks: Pool computes
    z = v*(w/2pi) + (b+pi)/2pi in fp16, DVE range-reduces with m = z mod 1,
    ACT evaluates sin(2pi*m - pi) -> f32r feats.  Num chunks: Pool affine,
    DVE prelu -> fp8 feats.
  * L1: sin K-blocks (4) in f32r, num K-blocks (3) in fp8e4 DoubleRow
    (weights split hi+lo across the two k-tiles so only the activation is
    single-quantized; measured full-batch max rel err 1.4e-2 < 2e-2).
    All W1 pre-scaled x16 to keep fp8 weights out of subnormals; the ACT
    eviction applies scale 1/16 and the per-partition b1 bias + leaky.
  * L2 in f32r.  b2 is pre-filled into the 4-bank PSUM tile by one DVE
    copy (start=False matmuls accumulate on top); ONE merged ACT prelu
    eviction [128, 2048] and ONE merged output DMA per tile.
  * Pipeline runs one tile skewed (enc(t+1) before mlp(t)).
"""

import numpy as np

import concourse.bass as bass
import concourse.tile as tile
import concourse.mybir as mybir
from concourse import bacc
from concourse.bass_utils import run_bass_kernel_spmd

B = 131072
N_CORES = 8
BC = B // N_CORES          # 16384 rows per core
PED = 256
NED = 128
CED = 512
Q = PED // 4               # 64
FEAT = 2 * PED + 3 * NED   # 896
NB = 512                   # batch columns per matmul tile
N_TILES = BC // NB         # 32
NSIN = 4                   # sin K-blocks (f32r)
NNUM = 3                   # num K-blocks (fp8 DoubleRow)
MC = CED // 128            # 4 output chunks

TWO_PI = 2.0 * np.pi
SW = 16.0                  # W1 scale (fp8 subnormal avoidance)

F32 = mybir.dt.float32
F32R = mybir.dt.float32r
F16 = mybir.dt.float16
F8 = mybir.dt.float8e4

# sin range reduction: mod-path (1 DVE op) or magic-round path (2 DVE ops)
USE_MOD = False
# fp8 DoubleRow for num blocks (else f32r everywhere)
NUM_DR = False
# stride-0 ktile dim on the DR moving operand (else duplicate feats8)
RHS_STRIDE0 = True


def _build_bass():
    nc = bacc.Bacc(
        "TRN2",
        target_bir_lowering=False,
        debug=False,
        enable_asserts=False,
        num_devices=N_CORES,
    )

    a8h = nc.dram_tensor("a8h", [8, BC], F16, kind="ExternalInput").ap()
    w1f = nc.dram_tensor("w1f", [128, NSIN * CED], F32R, kind="ExternalInput").ap()
    if NUM_DR:
        w1q = nc.dram_tensor("w1q", [128, NNUM * 2 * CED], F8,
                             kind="ExternalInput").ap()
    else:
        w1q = nc.dram_tensor("w1q", [128, NNUM * CED], F32R,
                             kind="ExternalInput").ap()
    w2s = nc.dram_tensor("w2s", [128, MC * CED], F32R, kind="ExternalInput").ap()
    b1s = nc.dram_tensor("b1s", [128, MC], F32, kind="ExternalInput").ap()
    b2b = nc.dram_tensor("b2b", [128, CED], F32, kind="ExternalInput").ap()
    pwb = nc.dram_tensor("pwb", [128, 10], F32, kind="ExternalInput").ap()
    nwb = nc.dram_tensor("nwb", [128, 6], F32, kind="ExternalInput").ap()
    out = nc.dram_tensor("out", [BC, CED], F32, kind="ExternalOutput").ap()

    with tile.TileContext(nc) as tc:
        with (
            tc.tile_pool(name="consts", bufs=1) as consts,
            tc.tile_pool(name="vbp", bufs=2) as vb_pool,
            tc.tile_pool(name="zp", bufs=2) as z_pool,
            tc.tile_pool(name="mp", bufs=2) as m_pool,
            tc.tile_pool(name="fsin", bufs=2) as fsin_pool,
            tc.tile_pool(name="fnum", bufs=2) as fnum_pool,
            tc.tile_pool(name="hp", bufs=2) as h_pool,
            tc.tile_pool(name="outp", bufs=3) as out_pool,
            tc.tile_pool(name="l1_ps", bufs=4, space="PSUM") as l1_psum,
            tc.tile_pool(name="l2_ps", bufs=1, space="PSUM") as l2_psum,
        ):
            w1f_sb = consts.tile([128, NSIN * CED], F32R)
            nc.sync.dma_start(out=w1f_sb[:], in_=w1f[:, :])
            if NUM_DR:
                w1q_sb = consts.tile([128, NNUM * 2 * CED], F8)
            else:
                w1q_sb = consts.tile([128, NNUM * CED], F32R)
            nc.sync.dma_start(out=w1q_sb[:], in_=w1q[:, :])
            w2_sb = consts.tile([128, MC * CED], F32R)
            nc.sync.dma_start(out=w2_sb[:], in_=w2s[:, :])
            b1_sb = consts.tile([128, MC], F32)
            nc.sync.dma_start(out=b1_sb[:], in_=b1s[:, :])
            b2b_sb = consts.tile([128, CED], F32)
            nc.sync.dma_start(out=b2b_sb[:], in_=b2b[:, :])
            pwb_sb = consts.tile([128, 10], F32)
            nc.sync.dma_start(out=pwb_sb[:], in_=pwb[:, :])
            nwb_sb = consts.tile([128, 6], F32)
            nc.sync.dma_start(out=nwb_sb[:], in_=nwb[:, :])

            state = {}

            def emit_enc(t):
                # one broadcast DMA: vb[p, c*512+n] = a8h[c, t*512+n]
                vb = vb_pool.tile([128, 7 * NB], F16, name=f"vb_{t}", tag="vb")
                src = bass.AP(
                    tensor=a8h.tensor, offset=t * NB,
                    ap=[[0, 128], [BC, 7], [1, NB]],
                )
                nc.sync.dma_start(out=vb[:], in_=src)

                fsin = fsin_pool.tile([128, NSIN * NB], F32R,
                                      name=f"fs_{t}", tag="fs")
                z = z_pool.tile([128, NSIN * NB], F16, name=f"z_{t}", tag="z")
                for c in range(NSIN):
                    nc.gpsimd.tensor_scalar(
                        z[:, c * NB:(c + 1) * NB], vb[:, c * NB:(c + 1) * NB],
                        pwb_sb[:, 2 * c:2 * c + 1],
                        pwb_sb[:, 2 * c + 1:2 * c + 2],
                        op0=mybir.AluOpType.mult, op1=mybir.AluOpType.add,
                    )
                # k' = fp16(z + 1536): the fp16 output cast rounds to the
                # nearest integer (ulp(1536) = 1).  Then
                # y = (z + 1536) - k' = z - round(z) in [-0.5, 0.5], computed
                # exactly in the f32 ALU.  sin(2*pi*y) = sin(2*pi*z).
                k = m_pool.tile([128, NSIN * NB], F16, name=f"k_{t}", tag="k")
                nc.vector.tensor_scalar(
                    k[:], z[:], 1536.0, 0.0,
                    op0=mybir.AluOpType.add, op1=mybir.AluOpType.bypass,
                )
                y = m_pool.tile([128, NSIN * NB], F16, name=f"y_{t}", tag="y")
                nc.vector.scalar_tensor_tensor(
                    y[:], z[:], 1536.0, k[:],
                    op0=mybir.AluOpType.add, op1=mybir.AluOpType.subtract,
                )
                nc.scalar.activation(
                    fsin[:], y[:], mybir.ActivationFunctionType.Sin,
                    scale=TWO_PI, bias=pwb_sb[:, 9:10],
                )

                fnum = fnum_pool.tile([128, NNUM * NB], F8 if NUM_DR else F32R,
                                      name=f"fn_{t}", tag="fn")
                for i in range(NNUM):
                    nc.scalar.activation(
                        fnum[:, i * NB:(i + 1) * NB],
                        vb[:, (NSIN + i) * NB:(NSIN + i + 1) * NB],
                        mybir.ActivationFunctionType.Prelu,
                        scale=nwb_sb[:, 2 * i:2 * i + 1],
                        bias=nwb_sb[:, 2 * i + 1:2 * i + 2],
                        alpha=0.01,
                    )
                state[t] = (fsin, fnum)

            def emit_mlp(t):
                fsin, fnum = state.pop(t)
                h = h_pool.tile([128, MC * NB], F32R, name=f"h_{t}", tag="h")
                for mci in range(MC):
                    l1p = l1_psum.tile([128, NB], F32, name=f"l1p_{t}_{mci}",
                                       tag="l1p")
                    nmm = NSIN + NNUM
                    for c in range(NSIN):
                        nc.tensor.matmul(
                            l1p[:],
                            w1f_sb[:, c * CED + mci * 128: c * CED + (mci + 1) * 128],
                            fsin[:, c * NB:(c + 1) * NB],
                            start=(c == 0), stop=False,
                        )
                    for i in range(NNUM):
                        last = (i == NNUM - 1)
                        if NUM_DR:
                            # lhsT [128, 2, 128]: ktile0=hi, ktile1=lo planes
                            lt = w1q_sb[:, (2 * i) * CED:(2 * i + 2) * CED].rearrange(
                                "p (two m) -> p two m", two=2
                            )[:, :, mci * 128:(mci + 1) * 128]
                            # rhs [128, 2, 512]: same fp8 feats in both k-tiles
                            rt = fnum[:, i * NB:(i + 1) * NB].unsqueeze(1)
                            rt = rt.broadcast_to([128, 2, NB])
                            nc.tensor.matmul(
                                l1p[:], lt, rt,
                                start=False, stop=last,
                                perf_mode=mybir.MatmulPerfMode.DoubleRow,
                            )
                        else:
                            nc.tensor.matmul(
                                l1p[:],
                                w1q_sb[:, i * CED + mci * 128:
                                       i * CED + (mci + 1) * 128],
                                fnum[:, i * NB:(i + 1) * NB],
                                start=False, stop=last,
                            )
                    nc.scalar.activation(
                        h[:, mci * NB:(mci + 1) * NB], l1p[:],
                        mybir.ActivationFunctionType.Prelu,
                        bias=b1_sb[:, mci:mci + 1], scale=1.0 / SW, alpha=0.01,
                    )

                # L2: 4-bank psum, b2 prefilled, one merged eviction + DMA
                l2p = l2_psum.tile([128, MC * NB], F32, name=f"l2p_{t}", tag="l2p")
                b2rep = b2b_sb.unsqueeze(1).broadcast_to([128, MC, NB])
                l2p3 = l2p.rearrange("p (m n) -> p m n", m=MC)
                nc.vector.tensor_scalar_add(l2p3, b2rep, 0.0)
                for j in range(MC):
                    for k in range(MC):
                        nc.tensor.matmul(
                            l2p[:, j * NB:(j + 1) * NB],
                            h[:, k * NB + j * 128: k * NB + (j + 1) * 128],
                            w2_sb[:, k * CED:(k + 1) * CED],
                            start=False, stop=(k == MC - 1),
                            skip_group_check=True,
                        )
                osb = out_pool.tile([128, MC * NB], F32, name=f"osb_{t}", tag="osb")
                nc.scalar.activation(
                    osb[:], l2p[:], mybir.ActivationFunctionType.Prelu,
                    bias=pwb_sb[:, 9:10], alpha=0.01,
                )
                dst = out[t * NB:(t + 1) * NB, :].rearrange("(j p) n -> p j n", j=MC)
                src = osb.rearrange("p (j n) -> p j n", j=MC)
                nc.sync.dma_start(out=dst, in_=src)

            emit_enc(0)
            for t in range(1, N_TILES):
                emit_enc(t)
                emit_mlp(t - 1)
            emit_mlp(N_TILES - 1)

    nc.compile()
    return nc


def _host_pack(inputs):
    f32 = lambda k: np.ascontiguousarray(np.asarray(inputs[k], dtype=np.float32))
    src = f32("src_xy")
    dst = f32("dst_xy")

    a8h = np.zeros((8, B), np.float16)
    a8h[0] = src[:, 0]
    a8h[1] = src[:, 1]
    a8h[2] = dst[:, 0]
    a8h[3] = dst[:, 1]
    a8h[4] = f32("time_s")
    a8h[5] = f32("wait_src")
    a8h[6] = f32("wait_dst")

    # sin params: z = v*(w/2pi) + boff; mod path: boff = (b+pi)/2pi and
    # sin(2pi*m - pi); magic path: boff = b/2pi and sin(2pi*y).
    pwb = np.zeros((128, 10), np.float32)
    pwb[:, 8] = -np.pi
    bshift = np.pi if USE_MOD else 0.0
    for c, (pfx, ax) in enumerate((("src", "x"), ("src", "y"),
                                   ("dst", "x"), ("dst", "y"))):
        pwb[:64, 2 * c] = f32(f"{pfx}_ws{ax}") / TWO_PI
        pwb[:64, 2 * c + 1] = (f32(f"{pfx}_bs{ax}") + bshift) / TWO_PI
        pwb[64:, 2 * c] = f32(f"{pfx}_wc{ax}") / TWO_PI
        pwb[64:, 2 * c + 1] = (f32(f"{pfx}_bc{ax}") + np.pi / 2 + bshift) / TWO_PI
    nwb = np.empty((128, 6), np.float32)
    for i, pfx in enumerate(("t", "ws", "wd")):
        nwb[:, 2 * i] = f32(f"{pfx}_w")
        nwb[:, 2 * i + 1] = f32(f"{pfx}_b")

    W1 = f32("W1") * SW                      # [896, 512], scaled
    w1f = np.empty((128, NSIN * CED), np.float32)
    for c in range(NSIN):
        w1f[:, c * CED:(c + 1) * CED] = W1[c * 128:(c + 1) * 128]
    if NUM_DR:
        import ml_dtypes
        E4 = getattr(ml_dtypes, "float8_e4m3fn", None) or ml_dtypes.float8_e4m3
        w1q = np.empty((128, NNUM * 2 * CED), E4)
        for i in range(NNUM):
            blk = W1[(NSIN + i) * 128:(NSIN + i + 1) * 128]
            hi = blk.astype(E4)
            lo = (blk - hi.astype(np.float32)).astype(E4)
            w1q[:, (2 * i) * CED:(2 * i + 1) * CED] = hi
            w1q[:, (2 * i + 1) * CED:(2 * i + 2) * CED] = lo
    else:
        w1q = np.empty((128, NNUM * CED), np.float32)
        for i in range(NNUM):
            w1q[:, i * CED:(i + 1) * CED] = W1[(NSIN + i) * 128:(NSIN + i + 1) * 128]

    W2 = f32("W2")
    w2s = np.empty((128, MC * CED), np.float32)
    for k in range(MC):
        w2s[:, k * CED:(k + 1) * CED] = W2[k * 128:(k + 1) * 128]
    b1s = f32("b1").reshape(MC, 128).T.copy()          # b1s[p, m] = b1[m*128+p]
    b2b = np.broadcast_to(f32("b2"), (128, CED)).copy()
    return a8h, pwb, nwb, w1f, w1q, w2s, b1s, b2b


_NC_CACHE = []


def kernel(**inputs) -> np.ndarray:
    a8h, pwb, nwb, w1f, w1q, w2s, b1s, b2b = _host_pack(inputs)

    if not _NC_CACHE:
        _NC_CACHE.append(_build_bass())
    nc = _NC_CACHE[0]

    in_maps = []
    for i in range(N_CORES):
        in_maps.append({
            "a8h": np.ascontiguousarray(a8h[:, i * BC:(i + 1) * BC]),
            "pwb": pwb, "nwb": nwb,
            "w1f": w1f, "w1q": w1q, "w2s": w2s,
            "b1s": b1s, "b2b": b2b,
        })

    res = run_bass_kernel_spmd(nc, in_maps, core_ids=list(range(N_CORES)))
    return np.concatenate([r["out"] for r in res.results], axis=0)


# revision 13
# speedup vs baseline: 1.0636x; 1.0636x over previous
"""Trainium2 Bass kernel for nn_ClaimEncoder (dense_mlp).

Math (per row):
  feats = [sin/cos point-encoders (2x256), leaky number-encoders (3x128)]  -> [896]
  h   = leaky_relu(feats @ W1 + b1)   -> [512]
  out = leaky_relu(h @ W2 + b2)       -> [512]

Strategy: pure data parallel over 8 NeuronCores (16384 rows each).

Device-side design (per core, batch tiles of NB=512 columns):
  * Encoder inputs arrive via ONE fp16 partition-broadcast DMA per tile
    (all 7 value rows at once).  Sin chunks: Pool computes
    z = v*(w/2pi) + (b+pi)/2pi in fp16, DVE range-reduces with m = z mod 1,
    ACT evaluates sin(2pi*m - pi) -> f32r feats.  Num chunks: Pool affine,
    DVE prelu -> fp8 feats.
  * L1: sin K-blocks (4) in f32r, num K-blocks (3) in fp8e4 DoubleRow
    (weights split hi+lo across the two k-tiles so only the activation is
    single-quantized; measured full-batch max rel err 1.4e-2 < 2e-2).
    All W1 pre-scaled x16 to keep fp8 weights out of subnormals; the ACT
    eviction applies scale 1/16 and the per-partition b1 bias + leaky.
  * L2 in f32r.  b2 is pre-filled into the 4-bank PSUM tile by one DVE
    copy (start=False matmuls accumulate on top); ONE merged ACT prelu
    eviction [128, 2048] and ONE merged output DMA per tile.
  * Pipeline runs one tile skewed (enc(t+1) before mlp(t)).
"""

import numpy as np

import concourse.bass as bass
import concourse.tile as tile
import concourse.mybir as mybir
from concourse import bacc
from concourse.bass_utils import run_bass_kernel_spmd

B = 131072
N_CORES = 8
BC = B // N_CORES          # 16384 rows per core
PED = 256
NED = 128
CED = 512
Q = PED // 4               # 64
FEAT = 2 * PED + 3 * NED   # 896
NB = 512                   # batch columns per matmul tile
N_TILES = BC // NB         # 32
NSIN = 4                   # sin K-blocks (f32r)
NNUM = 3                   # num K-blocks (fp8 DoubleRow)
MC = CED // 128            # 4 output chunks

TWO_PI = 2.0 * np.pi
SW = 16.0                  # W1 scale (fp8 subnormal avoidance)

F32 = mybir.dt.float32
F32R = mybir.dt.float32r
F16 = mybir.dt.float16
F8 = mybir.dt.float8e4

# sin range reduction: mod-path (1 DVE op) or magic-round path (2 DVE ops)
USE_MOD = False
# fp8 DoubleRow for num blocks (else f32r everywhere)
NUM_DR = True
# stride-0 ktile dim on the DR moving operand (else duplicate feats8)
RHS_STRIDE0 = True


def _build_bass():
    nc = bacc.Bacc(
        "TRN2",
        target_bir_lowering=False,
        debug=False,
        enable_asserts=False,
        num_devices=N_CORES,
    )

    a8h = nc.dram_tensor("a8h", [8, BC], F16, kind="ExternalInput").ap()
    w1f = nc.dram_tensor("w1f", [128, NSIN * CED], F32R, kind="ExternalInput").ap()
    if NUM_DR:
        w1q = nc.dram_tensor("w1q", [128, NNUM * 2 * CED], F8,
                             kind="ExternalInput").ap()
    else:
        w1q = nc.dram_tensor("w1q", [128, NNUM * CED], F32R,
                             kind="ExternalInput").ap()
    w2s = nc.dram_tensor("w2s", [128, MC * CED], F32R, kind="ExternalInput").ap()
    b1s = nc.dram_tensor("b1s", [128, MC], F32, kind="ExternalInput").ap()
    b2b = nc.dram_tensor("b2b", [128, MC * CED], F32, kind="ExternalInput").ap()
    pwb = nc.dram_tensor("pwb", [128, 10], F32, kind="ExternalInput").ap()
    nwb = nc.dram_tensor("nwb", [128, 6], F32, kind="ExternalInput").ap()
    out = nc.dram_tensor("out", [BC, CED], F32, kind="ExternalOutput").ap()

    with tile.TileContext(nc) as tc:
        with (
            tc.tile_pool(name="consts", bufs=1) as consts,
            tc.tile_pool(name="vbp", bufs=2) as vb_pool,
            tc.tile_pool(name="zp", bufs=2) as z_pool,
            tc.tile_pool(name="mp", bufs=2) as m_pool,
            tc.tile_pool(name="fsin", bufs=2) as fsin_pool,
            tc.tile_pool(name="fnum", bufs=2) as fnum_pool,
            tc.tile_pool(name="hp", bufs=2) as h_pool,
            tc.tile_pool(name="outp", bufs=3) as out_pool,
            tc.tile_pool(name="l2s", bufs=2) as l2s_pool,
            tc.tile_pool(name="l1_ps", bufs=4, space="PSUM") as l1_psum,
            tc.tile_pool(name="l2_ps", bufs=1, space="PSUM") as l2_psum,
        ):
            w1f_sb = consts.tile([128, NSIN * CED], F32R)
            nc.sync.dma_start(out=w1f_sb[:], in_=w1f[:, :])
            if NUM_DR:
                w1q_sb = consts.tile([128, NNUM * 2 * CED], F8)
            else:
                w1q_sb = consts.tile([128, NNUM * CED], F32R)
            nc.sync.dma_start(out=w1q_sb[:], in_=w1q[:, :])
            w2_sb = consts.tile([128, MC * CED], F32R)
            nc.sync.dma_start(out=w2_sb[:], in_=w2s[:, :])
            b1_sb = consts.tile([128, MC], F32)
            nc.sync.dma_start(out=b1_sb[:], in_=b1s[:, :])
            b2b_sb = consts.tile([128, MC * CED], F32)
            nc.sync.dma_start(out=b2b_sb[:], in_=b2b[:, :])
            pwb_sb = consts.tile([128, 10], F32)
            nc.sync.dma_start(out=pwb_sb[:], in_=pwb[:, :])
            nwb_sb = consts.tile([128, 6], F32)
            nc.sync.dma_start(out=nwb_sb[:], in_=nwb[:, :])

            state = {}

            def emit_enc(t):
                # one broadcast DMA: vb[p, c*512+n] = a8h[c, t*512+n]
                vb = vb_pool.tile([128, 7 * NB], F16, name=f"vb_{t}", tag="vb")
                src = bass.AP(
                    tensor=a8h.tensor, offset=t * NB,
                    ap=[[0, 128], [BC, 7], [1, NB]],
                )
                nc.sync.dma_start(out=vb[:], in_=src)

                fsin = fsin_pool.tile([128, NSIN * NB], F32R,
                                      name=f"fs_{t}", tag="fs")
                z = z_pool.tile([128, NSIN * NB], F16, name=f"z_{t}", tag="z")
                for c in range(NSIN):
                    nc.gpsimd.tensor_scalar(
                        z[:, c * NB:(c + 1) * NB], vb[:, c * NB:(c + 1) * NB],
                        pwb_sb[:, 2 * c:2 * c + 1],
                        pwb_sb[:, 2 * c + 1:2 * c + 2],
                        op0=mybir.AluOpType.mult, op1=mybir.AluOpType.add,
                    )
                # k' = fp16(z + 1536): the fp16 output cast rounds to the
                # nearest integer (ulp(1536) = 1).  Then
                # y = (z + 1536) - k' = z - round(z) in [-0.5, 0.5], computed
                # exactly in the f32 ALU.  sin(2*pi*y) = sin(2*pi*z).
                k = m_pool.tile([128, NSIN * NB], F16, name=f"k_{t}", tag="k")
                nc.vector.tensor_scalar(
                    k[:], z[:], 1536.0, 0.0,
                    op0=mybir.AluOpType.add, op1=mybir.AluOpType.bypass,
                )
                y = m_pool.tile([128, NSIN * NB], F16, name=f"y_{t}", tag="y")
                nc.vector.scalar_tensor_tensor(
                    y[:], z[:], 1536.0, k[:],
                    op0=mybir.AluOpType.add, op1=mybir.AluOpType.subtract,
                )
                nc.scalar.activation(
                    fsin[:], y[:], mybir.ActivationFunctionType.Sin,
                    scale=TWO_PI, bias=pwb_sb[:, 9:10],
                )

                fnum = fnum_pool.tile([128, NNUM * NB], F8 if NUM_DR else F32R,
                                      name=f"fn_{t}", tag="fn")
                for i in range(NNUM):
                    nc.scalar.activation(
                        fnum[:, i * NB:(i + 1) * NB],
                        vb[:, (NSIN + i) * NB:(NSIN + i + 1) * NB],
                        mybir.ActivationFunctionType.Prelu,
                        scale=nwb_sb[:, 2 * i:2 * i + 1],
                        bias=nwb_sb[:, 2 * i + 1:2 * i + 2],
                        alpha=0.01,
                    )
                state[t] = (fsin, fnum)

            def emit_mlp(t):
                fsin, fnum = state.pop(t)
                h = h_pool.tile([128, MC * NB], F32R, name=f"h_{t}", tag="h")
                for mci in range(MC):
                    l1p = l1_psum.tile([128, NB], F32, name=f"l1p_{t}_{mci}",
                                       tag="l1p")
                    nmm = NSIN + NNUM
                    for c in range(NSIN):
                        nc.tensor.matmul(
                            l1p[:],
                            w1f_sb[:, c * CED + mci * 128: c * CED + (mci + 1) * 128],
                            fsin[:, c * NB:(c + 1) * NB],
                            start=(c == 0), stop=False,
                        )
                    for i in range(NNUM):
                        last = (i == NNUM - 1)
                        if NUM_DR:
                            # lhsT [128, 2, 128]: ktile0=hi, ktile1=lo planes
                            lt = w1q_sb[:, (2 * i) * CED:(2 * i + 2) * CED].rearrange(
                                "p (two m) -> p two m", two=2
                            )[:, :, mci * 128:(mci + 1) * 128]
                            # rhs [128, 2, 512]: same fp8 feats in both k-tiles
                            rt = fnum[:, i * NB:(i + 1) * NB].unsqueeze(1)
                            rt = rt.broadcast_to([128, 2, NB])
                            nc.tensor.matmul(
                                l1p[:], lt, rt,
                                start=False, stop=last,
                                perf_mode=mybir.MatmulPerfMode.DoubleRow,
                            )
                        else:
                            nc.tensor.matmul(
                                l1p[:],
                                w1q_sb[:, i * CED + mci * 128:
                                       i * CED + (mci + 1) * 128],
                                fnum[:, i * NB:(i + 1) * NB],
                                start=False, stop=last,
                            )
                    nc.scalar.activation(
                        h[:, mci * NB:(mci + 1) * NB], l1p[:],
                        mybir.ActivationFunctionType.Prelu,
                        bias=b1_sb[:, mci:mci + 1], scale=1.0 / SW, alpha=0.01,
                    )

                l2p = l2_psum.tile([128, MC * NB], F32, name=f"l2p_{t}", tag="l2p")
                for j in range(MC):
                    for k in range(MC):
                        nc.tensor.matmul(
                            l2p[:, j * NB:(j + 1) * NB],
                            h[:, k * NB + j * 128: k * NB + (j + 1) * 128],
                            w2_sb[:, k * CED:(k + 1) * CED],
                            start=(k == 0), stop=(k == MC - 1),
                        )
                # add b2 (free-dim vector) on DVE, then leaky on ACT
                s2 = l2s_pool.tile([128, MC * NB], F32, name=f"s2_{t}", tag="s2")
                nc.vector.scalar_tensor_tensor(
                    s2[:], l2p[:], 1.0, b2b_sb[:],
                    op0=mybir.AluOpType.mult, op1=mybir.AluOpType.add,
                )
                osb = out_pool.tile([128, MC * NB], F32, name=f"osb_{t}", tag="osb")
                nc.scalar.activation(
                    osb[:], s2[:], mybir.ActivationFunctionType.Prelu,
                    bias=pwb_sb[:, 9:10], alpha=0.01,
                )
                dst = out[t * NB:(t + 1) * NB, :].rearrange("(j p) n -> p j n", j=MC)
                nc.sync.dma_start(out=dst, in_=osb[:])

            emit_enc(0)
            for t in range(1, N_TILES):
                emit_enc(t)
                emit_mlp(t - 1)
            emit_mlp(N_TILES - 1)

    nc.compile()
    return nc


def _host_pack(inputs):
    f32 = lambda k: np.ascontiguousarray(np.asarray(inputs[k], dtype=np.float32))
    src = f32("src_xy")
    dst = f32("dst_xy")

    a8h = np.zeros((8, B), np.float16)
    a8h[0] = src[:, 0]
    a8h[1] = src[:, 1]
    a8h[2] = dst[:, 0]
    a8h[3] = dst[:, 1]
    a8h[4] = f32("time_s")
    a8h[5] = f32("wait_src")
    a8h[6] = f32("wait_dst")

    # sin params: z = v*(w/2pi) + boff; mod path: boff = (b+pi)/2pi and
    # sin(2pi*m - pi); magic path: boff = b/2pi and sin(2pi*y).
    pwb = np.zeros((128, 10), np.float32)
    pwb[:, 8] = -np.pi
    bshift = np.pi if USE_MOD else 0.0
    for c, (pfx, ax) in enumerate((("src", "x"), ("src", "y"),
                                   ("dst", "x"), ("dst", "y"))):
        pwb[:64, 2 * c] = f32(f"{pfx}_ws{ax}") / TWO_PI
        pwb[:64, 2 * c + 1] = (f32(f"{pfx}_bs{ax}") + bshift) / TWO_PI
        pwb[64:, 2 * c] = f32(f"{pfx}_wc{ax}") / TWO_PI
        pwb[64:, 2 * c + 1] = (f32(f"{pfx}_bc{ax}") + np.pi / 2 + bshift) / TWO_PI
    nwb = np.empty((128, 6), np.float32)
    for i, pfx in enumerate(("t", "ws", "wd")):
        nwb[:, 2 * i] = f32(f"{pfx}_w")
        nwb[:, 2 * i + 1] = f32(f"{pfx}_b")

    W1 = f32("W1") * SW                      # [896, 512], scaled
    w1f = np.empty((128, NSIN * CED), np.float32)
    for c in range(NSIN):
        w1f[:, c * CED:(c + 1) * CED] = W1[c * 128:(c + 1) * 128]
    if NUM_DR:
        import ml_dtypes
        E4 = getattr(ml_dtypes, "float8_e4m3fn", None) or ml_dtypes.float8_e4m3
        w1q = np.empty((128, NNUM * 2 * CED), E4)
        for i in range(NNUM):
            blk = W1[(NSIN + i) * 128:(NSIN + i + 1) * 128]
            hi = blk.astype(E4)
            lo = (blk - hi.astype(np.float32)).astype(E4)
            w1q[:, (2 * i) * CED:(2 * i + 1) * CED] = hi
            w1q[:, (2 * i + 1) * CED:(2 * i + 2) * CED] = lo
    else:
        w1q = np.empty((128, NNUM * CED), np.float32)
        for i in range(NNUM):
            w1q[:, i * CED:(i + 1) * CED] = W1[(NSIN + i) * 128:(NSIN + i + 1) * 128]

    W2 = f32("W2")
    w2s = np.empty((128, MC * CED), np.float32)
    for k in range(MC):
        w2s[:, k * CED:(k + 1) * CED] = W2[k * 128:(k + 1) * 128]
    b1s = f32("b1").reshape(MC, 128).T.copy()          # b1s[p, m] = b1[m*128+p]
    b2b = np.tile(np.broadcast_to(f32("b2"), (128, CED)), (1, MC))
    return a8h, pwb, nwb, w1f, w1q, w2s, b1s, b2b


_NC_CACHE = []


def kernel(**inputs) -> np.ndarray:
    a8h, pwb, nwb, w1f, w1q, w2s, b1s, b2b = _host_pack(inputs)

    if not _NC_CACHE:
        _NC_CACHE.append(_build_bass())
    nc = _NC_CACHE[0]

    in_maps = []
    for i in range(N_CORES):
        in_maps.append({
            "a8h": np.ascontiguousarray(a8h[:, i * BC:(i + 1) * BC]),
            "pwb": pwb, "nwb": nwb,
            "w1f": w1f, "w1q": w1q, "w2s": w2s,
            "b1s": b1s, "b2b": b2b,
        })

    res = run_bass_kernel_spmd(nc, in_maps, core_ids=list(range(N_CORES)))
    return np.concatenate([r["out"] for r in res.results], axis=0)
